# revision 1
# baseline (speedup 1.0000x reference)
"""7x7 'same' 2D convolution over [128, 512, 512] f32, data-parallel on 8 NeuronCores.

Formulation: for each output-row block of M=122 rows, the row-direction
(u-tap) contraction is a banded Toeplitz matmul on the TensorEngine:
    out[i0+m, j] = sum_v sum_r T_v[r, m] * xpad[i0+r, j+v]
with T_v[r, m] = w[r-m, v] (band 0 <= r-m < 7). The 7 column taps (v)
are 7 matmuls accumulating into the same PSUM bank, each reading the
same SBUF x-tile at a shifted column offset. Inputs are cast to fp16
host-side (full-rate on the PE, ~1e-3 rel err); accumulation is fp32.
"""

import numpy as np

B, H, W = 128, 512, 512
KS = 7
PAD = (KS - 1) // 2          # 3
HP = H + 2 * PAD             # 518
N_CORES = 8
PER_CORE = B // N_CORES      # 16
MBLK = 128 - (KS - 1)        # 122 output rows per full block
# output-row blocks: (out_start, n_out_rows, n_in_rows)
BLOCKS = []
_o = 0
while _o < H:
    m = min(MBLK, H - _o)
    BLOCKS.append((_o, m, m + KS - 1))
    _o += m


def _build_program():
    import concourse.bass as bass
    import concourse.tile as tile
    from concourse import bacc, mybir

    f16 = mybir.dt.float16
    f32 = mybir.dt.float32

    nc = bacc.Bacc("TRN2", target_bir_lowering=False, debug=False,
                   num_devices=N_CORES)
    x_ext = nc.declare_dram_parameter("x", [PER_CORE, HP, HP], f16,
                                      isOutput=False)
    t_ext = nc.declare_dram_parameter("toep", [128, KS * MBLK], f16,
                                      isOutput=False)
    out_ext = nc.declare_dram_parameter("out", [PER_CORE, H, W], f32,
                                        isOutput=True)

    with tile.TileContext(nc) as tc:
        with (
            tc.tile_pool(name="toep", bufs=1) as toep_pool,
            tc.tile_pool(name="xin", bufs=4) as x_pool,
            tc.tile_pool(name="psum", bufs=4, space="PSUM") as psum_pool,
            tc.tile_pool(name="outs", bufs=4) as out_pool,
        ):
            toep_sb = toep_pool.tile([128, KS * MBLK], f16)
            nc.sync.dma_start(out=toep_sb[:], in_=t_ext[:])

            for img in range(PER_CORE):
                for (o0, m, kin) in BLOCKS:
                    x_tile = x_pool.tile([128, HP], f16)
                    nc.sync.dma_start(out=x_tile[:kin, :],
                                      in_=x_ext[img, o0:o0 + kin, :])
                    psum = psum_pool.tile([128, W], f32)
                    for v in range(KS):
                        nc.tensor.matmul(
                            psum[:m, :],
                            toep_sb[:kin, v * MBLK:v * MBLK + m],
                            x_tile[:kin, v:v + W],
                            start=(v == 0),
                            stop=(v == KS - 1),
                        )
                    o_sb = out_pool.tile([128, W], f32)
                    nc.vector.tensor_copy(o_sb[:m, :], psum[:m, :])
                    nc.sync.dma_start(out=out_ext[img, o0:o0 + m, :],
                                      in_=o_sb[:m, :])
    nc.finalize()
    return nc


def _host_prep(x, w):
    x = np.asarray(x, dtype=np.float32)
    w = np.asarray(w, dtype=np.float32)
    xpad = np.zeros((B, HP, HP), dtype=np.float16)
    xpad[:, PAD:PAD + H, PAD:PAD + W] = x
    toep = np.zeros((128, KS * MBLK), dtype=np.float16)
    w16 = w.astype(np.float16)
    idx = np.arange(MBLK)
    for v in range(KS):
        for d in range(KS):
            toep[idx + d, v * MBLK + idx] = w16[d, v]
    return xpad, toep


def kernel(x, w):
    from concourse.bass_utils import run_bass_kernel_spmd

    xpad, toep = _host_prep(x, w)
    nc = _build_program()
    in_maps = [
        {"x": xpad[c * PER_CORE:(c + 1) * PER_CORE], "toep": toep}
        for c in range(N_CORES)
    ]
    res = run_bass_kernel_spmd(nc, in_maps, core_ids=list(range(N_CORES)))
    return np.concatenate(
        [res.results[c]["out"] for c in range(N_CORES)], axis=0
    )
